# revision 41
# baseline (speedup 1.0000x reference)
"""CTBG circuit kernel for Trainium2, data-parallel over batch on 8 NeuronCores.

Network (per reference):
  gpe_out = x @ (gpe_w * gpe_mask.T) + gpe_b              [B, 1536]
  gpi_in  = concat([x, gpe_out], -1)                      [B, 3072]
  gpi_out = gpi_in @ (gpi_w * gpi_mask.T) + gpi_b         [B, 1536]
  h1 = relu(gpi_out @ w1 + b1); h2 = relu(h1 @ w2 + b2)
  out = relu(h2 @ w3 + b3)                                [B, 6]

Per-core dataflow (feature-major activations, bf16 compute, f32 accumulate):
  - Two concurrent DMA streams so the load phase runs near the HBM
    roofline instead of serializing on one queue:
      * SWDGE (gpsimd) casts x and all weights f32->bf16 in multi-MB
        transfers straight into the matmul-ready [128, k, n] layouts.
      * HWDGE (sync) streams the masks as raw f32 row-tiles.
  - Masks are PE-transposed (f32, identity matmul) into PSUM; DVE
    multiplies them into the resident bf16 weight tiles in place.
  - x row-chunks are PE-transposed to feature-major xT tiles (bf16).
  - Matmul chain keeps activations feature-major; ScalarE drains PSUM
    with bias (+relu for the MLP) straight to bf16 tiles feeding the
    next layer.
  - Output written as [6, 2048] f32 per core; host transposes + concats.
"""

import numpy as np

NCORES = 8
B = 16384
BS = B // NCORES          # 2048 rows per core
BT = 512                  # batch tile (matmul free dim)
NBT = BS // BT            # 4
D1 = 1536                 # gpe input dim
D2 = 1536                 # gpe output dim
D3 = 3072                 # gpi input dim
D4 = 1536                 # gpi output dim
H = 512                   # mlp hidden
A = 6                     # action dim

K1 = D1 // 128            # 12
U2 = D2 // 128            # 12
K3 = D3 // 128            # 24
V4 = D4 // 128            # 12
M5 = H // 128             # 4

_CACHE = {}


def _build():
    import concourse.bacc as bacc
    import concourse.tile as tile
    from concourse import mybir
    from concourse.masks import make_identity

    FP32 = mybir.dt.float32
    BF16 = mybir.dt.bfloat16
    Act = mybir.ActivationFunctionType

    nc = bacc.Bacc(None)

    x_d = nc.dram_tensor("x", [BS, D1], FP32, kind="ExternalInput")
    gpem_d = nc.dram_tensor("gpe_mask", [D2, D1], FP32, kind="ExternalInput")
    gpew_d = nc.dram_tensor("gpe_w", [D1, D2], FP32, kind="ExternalInput")
    gpeb_d = nc.dram_tensor("gpe_b", [D2], FP32, kind="ExternalInput")
    gpim_d = nc.dram_tensor("gpi_mask", [D4, D3], FP32, kind="ExternalInput")
    gpiw_d = nc.dram_tensor("gpi_w", [D3, D4], FP32, kind="ExternalInput")
    gpib_d = nc.dram_tensor("gpi_b", [D4], FP32, kind="ExternalInput")
    w1_d = nc.dram_tensor("w1", [D4, H], FP32, kind="ExternalInput")
    b1_d = nc.dram_tensor("b1", [H], FP32, kind="ExternalInput")
    w2_d = nc.dram_tensor("w2", [H, H], FP32, kind="ExternalInput")
    b2_d = nc.dram_tensor("b2", [H], FP32, kind="ExternalInput")
    w3_d = nc.dram_tensor("w3", [H, A], FP32, kind="ExternalInput")
    b3_d = nc.dram_tensor("b3", [A], FP32, kind="ExternalInput")
    o_d = nc.dram_tensor("out", [A, BS], FP32, kind="ExternalOutput")

    with tile.TileContext(nc) as tc:
        with (
            tc.tile_pool(name="wpool", bufs=1) as wp,        # persistent weights
            tc.tile_pool(name="mpool", bufs=2) as mp,        # mask row staging
            tc.tile_pool(name="xrpool", bufs=2) as xrp,      # x row-chunks bf16
            tc.tile_pool(name="xpool", bufs=1) as xp,        # xT feature-major
            tc.tile_pool(name="apool", bufs=1) as ap,        # activations
            tc.tile_pool(name="hpool", bufs=1) as hp,        # mlp activations
            tc.tile_pool(name="opool", bufs=2) as op,        # output staging
            tc.tile_pool(name="pspool", bufs=5, space="PSUM") as psp,
            tc.tile_pool(name="pstpool", bufs=2, space="PSUM") as pstp,
            tc.tile_pool(name="pstfpool", bufs=1, space="PSUM") as pstfp,
        ):
            # ------------- SWDGE (gpsimd): big bf16 cast loads ------------
            # FIFO order = consumption order; x tiles 1-3 go last so their
            # staging-ring WAR stalls cannot block the weight stream.
            def load_x_chunk(t_i, hh):
                t = xrp.tile([128, 2, D1], BF16, tag="xr")
                r0 = t_i * 4 + hh * 2
                nc.gpsimd.dma_start(
                    out=t[:, :, :],
                    in_=x_d[r0 * 128:(r0 + 2) * 128, :].rearrange(
                        "(r p) n -> p r n", p=128))
                return t

            xrow = {}
            wgpe = wp.tile([128, K1, D2], BF16, tag="wgpe")
            nc.gpsimd.dma_start(
                out=wgpe[:, :, :],
                in_=gpew_d.rearrange("(k p) n -> p k n", p=128))
            for hh in range(2):
                xrow[(0, hh)] = load_x_chunk(0, hh)

            wgpi = wp.tile([128, K3, D4], BF16, tag="wgpi")
            for c in range(2):
                nc.gpsimd.dma_start(
                    out=wgpi[:, c * (K3 // 2):(c + 1) * (K3 // 2), :],
                    in_=gpiw_d[c * (D3 // 2):(c + 1) * (D3 // 2), :].rearrange(
                        "(k p) n -> p k n", p=128))

            w1s = wp.tile([128, V4, H], BF16, tag="w1s")
            nc.gpsimd.dma_start(
                out=w1s[:, :, :],
                in_=w1_d.rearrange("(k p) n -> p k n", p=128))
            w2s = wp.tile([128, M5, H], BF16, tag="w2s")
            nc.gpsimd.dma_start(
                out=w2s[:, :, :],
                in_=w2_d.rearrange("(k p) n -> p k n", p=128))
            w3s = wp.tile([128, M5, A], BF16, tag="w3s")
            nc.gpsimd.dma_start(
                out=w3s[:, :, :],
                in_=w3_d.rearrange("(k p) a -> p k a", p=128))
            for t_i in range(1, NBT):
                for hh in range(2):
                    xrow[(t_i, hh)] = load_x_chunk(t_i, hh)

            # ------------- HWDGE (sync): masks (raw f32) + biases ---------
            ident = wp.tile([128, 128], FP32, tag="ident")
            make_identity(nc, ident[:, :])
            identb = wp.tile([128, 128], BF16, tag="identb")
            make_identity(nc, identb[:, :])

            def load_bias(b_dram, n, tag):
                nat = mp.tile([max(n, 1), 128], FP32, tag="bnat")
                nc.sync.dma_start(out=nat[:, :],
                                  in_=b_dram.rearrange("(c p) -> c p", p=128))
                ps = pstfp.tile([128, 128], FP32, tag="pstf")
                nc.tensor.transpose(ps[0:128, 0:n], nat[:, :], ident[0:n, 0:n])
                sb = wp.tile([128, max(n, 1)], FP32, tag=tag)
                nc.vector.tensor_copy(sb[:, 0:n], ps[0:128, 0:n])
                return sb

            gpeb_sb = load_bias(gpeb_d, U2, "gpeb")
            gpib_sb = load_bias(gpib_d, V4, "gpib")
            b1_sb = load_bias(b1_d, M5, "b1sb")
            b2_sb = load_bias(b2_d, M5, "b2sb")
            b3_sb = wp.tile([A, 1], FP32, tag="b3sb")
            nc.sync.dma_start(out=b3_sb[:, :],
                              in_=b3_d.rearrange("(a one) -> a one", one=1))

            # mask row-tiles: raw f32 rows staged through a small ring, then
            # DVE-cast to bf16 so the PE transposes get FWL (2x faster than
            # fp32 transpose mode); gpi rows arrive as two half-row DMAs.
            def load_mask_row(dram, u0, half):
                t = mp.tile([128, D1], FP32, tag="mrow")
                nc.sync.dma_start(
                    out=t[:, :],
                    in_=dram[u0 * 128:(u0 + 1) * 128,
                             half * D1:(half + 1) * D1])
                tb = mp.tile([128, D1], BF16, tag="mrowb")
                nc.vector.tensor_copy(tb[:, :], t[:, :])
                return tb

            # ------------- PE prep: mask transposes + x transposes --------
            def prep_mask_row(row, u0, c0, ncols, wtile):
                """transpose mask row [128, ncols*128] block by block and
                multiply into wtile[:, c0+c, u0-slice]."""
                for c in range(ncols):
                    ps = pstp.tile([128, 128], BF16, tag="pstb")
                    nc.tensor.transpose(ps[:, :],
                                        row[:, c * 128:(c + 1) * 128],
                                        identb[:, :])
                    nc.vector.tensor_mul(
                        wtile[:, c0 + c, u0 * 128:(u0 + 1) * 128],
                        wtile[:, c0 + c, u0 * 128:(u0 + 1) * 128],
                        ps[:, :])

            def prep_xT(t_i, xt):
                """x chunks [128, 2, D1] -> xT tile [128, K1, BT]."""
                for hh in range(2):
                    rows = xrow[(t_i, hh)]
                    for r in range(2):
                        g = hh * 2 + r
                        for c in range(K1):
                            ps = pstp.tile([128, 128], BF16, tag="pstb")
                            nc.tensor.transpose(
                                ps[:, :], rows[:, r, c * 128:(c + 1) * 128],
                                identb[:, :])
                            nc.scalar.activation(
                                xt[:, c, g * 128:(g + 1) * 128], ps[:, :],
                                Act.Copy)

            # gpe mask prep (stream 12 rows x 12 cols), then first xT
            for u0 in range(U2):
                row = load_mask_row(gpem_d, u0, 0)
                prep_mask_row(row, u0, 0, K1, wgpe)
            xT = xp.tile([128, K1, BT], BF16, tag="xT")
            prep_xT(0, xT)

            # ------------- main loop over batch tiles ---------------------
            gpe_out = ap.tile([128, U2, BT], BF16, tag="gpe_out")
            gpi_out = ap.tile([128, V4, BT], BF16, tag="gpi_out")

            for t_i in range(NBT):
                # L1: gpe_out[u] = sum_k mw_gpe[k,u] @ xT[k]  (+bias)
                for u in range(U2):
                    ps = psp.tile([128, BT], FP32, tag="ps")
                    for k in range(K1):
                        nc.tensor.matmul(ps[:, :],
                                         wgpe[:, k, u * 128:(u + 1) * 128],
                                         xT[:, k, :],
                                         start=(k == 0), stop=(k == K1 - 1))
                    nc.scalar.activation(gpe_out[:, u, :], ps[:, :],
                                         Act.Identity,
                                         bias=gpeb_sb[:, u:u + 1])
                    if t_i == 0:
                        # gpi masked-weight prep interleaved with L1 so the
                        # PE transposes and DVE muls overlap L1's matmuls
                        for half in range(2):
                            row = load_mask_row(gpim_d, u, half)
                            prep_mask_row(row, u, half * K1, K1, wgpi)

                # L2: gpi_out[v] = sum_k mw_gpi[k,v] @ gpi_in[k]  (+bias)
                for v in range(V4):
                    ps = psp.tile([128, BT], FP32, tag="ps")
                    for k in range(K3):
                        rhs = xT[:, k, :] if k < K1 else gpe_out[:, k - K1, :]
                        nc.tensor.matmul(ps[:, :],
                                         wgpi[:, k, v * 128:(v + 1) * 128],
                                         rhs,
                                         start=(k == 0), stop=(k == K3 - 1))
                    nc.scalar.activation(gpi_out[:, v, :], ps[:, :],
                                         Act.Identity,
                                         bias=gpib_sb[:, v:v + 1])

                # next tile's xT (single buffer: xT-t frees after L2-t)
                if t_i + 1 < NBT:
                    prep_xT(t_i + 1, xT)

                # L3: h1 = relu(gpi_out @ w1 + b1)
                h1 = hp.tile([128, M5, BT], BF16, tag="h1")
                for m in range(M5):
                    ps = psp.tile([128, BT], FP32, tag="ps")
                    for k in range(V4):
                        nc.tensor.matmul(ps[:, :],
                                         w1s[:, k, m * 128:(m + 1) * 128],
                                         gpi_out[:, k, :],
                                         start=(k == 0), stop=(k == V4 - 1))
                    nc.scalar.activation(h1[:, m, :], ps[:, :], Act.Relu,
                                         bias=b1_sb[:, m:m + 1])

                # L4: h2 = relu(h1 @ w2 + b2)
                h2 = hp.tile([128, M5, BT], BF16, tag="h2")
                for m in range(M5):
                    ps = psp.tile([128, BT], FP32, tag="ps")
                    for k in range(M5):
                        nc.tensor.matmul(ps[:, :],
                                         w2s[:, k, m * 128:(m + 1) * 128],
                                         h1[:, k, :],
                                         start=(k == 0), stop=(k == M5 - 1))
                    nc.scalar.activation(h2[:, m, :], ps[:, :], Act.Relu,
                                         bias=b2_sb[:, m:m + 1])

                # L5: out = relu(h2 @ w3 + b3), [6, BT] f32
                ps5f = psp.tile([128, BT], FP32, tag="ps")
                ps5 = ps5f[0:A, :]
                for k in range(M5):
                    nc.tensor.matmul(ps5[:, :], w3s[:, k, :], h2[:, k, :],
                                     start=(k == 0), stop=(k == M5 - 1))
                osb = op.tile([A, BT], FP32, tag="osb")
                nc.scalar.activation(osb[:, :], ps5[:, :], Act.Relu,
                                     bias=b3_sb[:, 0:1])
                nc.sync.dma_start(out=o_d[:, t_i * BT:(t_i + 1) * BT],
                                  in_=osb[:, :])

    nc.finalize()
    return nc


def _get_nc():
    if "nc" not in _CACHE:
        _CACHE["nc"] = _build()
    return _CACHE["nc"]


def _run(inputs, trace=False):
    from concourse.bass_utils import run_bass_kernel_spmd

    nc = _get_nc()
    shared = {k: np.ascontiguousarray(v, dtype=np.float32)
              for k, v in inputs.items() if k != "x"}
    x = np.ascontiguousarray(inputs["x"], dtype=np.float32)
    in_maps = [dict(shared, x=x[c * BS:(c + 1) * BS]) for c in range(NCORES)]
    res = run_bass_kernel_spmd(nc, in_maps, list(range(NCORES)), trace=trace)
    out = np.concatenate(
        [np.asarray(res.results[c]["out"]).T for c in range(NCORES)], axis=0)
    return out.astype(np.float32), res


def kernel(**inputs):
    out, _ = _run(inputs, trace=False)
    return out
